# revision 4
# baseline (speedup 1.0000x reference)
"""GAT_TS2 Trainium2 kernel: 8-core SPMD, i-row-sharded with per-layer AllGather.

Sharding: core c owns batch c//4, rows 256*(c%4) .. 256*(c%4)+256 (the "i" rows of
the [B,N,N] attention). Per attention layer each core computes its 256 rows of the
row-softmax and the bij,bjf contraction locally against the full gathered hidden
state; the updated h1 slices are exchanged with one 4-core AllGather per layer
(two layers per GRU step, 31 steps).

Device math layout (all fp32):
  - state kept transposed ("T-layout"): [65, 256] = [H+ones_row, own_i]
  - s_src/s_dst rows from one rank-2 matmul against augmented weights
  - score tile s[j,i] = s_src[j]+s_dst[i] built by rank-2 PE matmuls into PSUM
    (lhsT = (s_src_block; ones), rhs = (ones; s_dst)) -> [128, 8, 256] psum
  - leaky-relu via ACT Prelu(alpha=.01), exp via ACT Exp -- both live in the
    "exp_and_others" table set, as does Tanh; sigmoid is computed as
    1/(1+exp(-x)) so the ACT table never swaps.
  - attention matmul: lhsT = gathered h1 in normal layout [128j, 65] (65th col
    = ones -> row 64 of the psum accumulates the softmax denominator), rhs =
    exp tile [128j, 256i], accumulated over the 8 j-blocks.
  - update: newh1T = att * (ones @ recip(den)) + h1T  (PE broadcasts the
    per-column reciprocal across partitions).

Host does the tiny fc_in projection, weight augmentation, and the final FFN head.
"""

import os
import sys

import numpy as np

B, T, N, F, H = 2, 32, 1024, 6, 64
SLOPE = 0.01
NCORES = 8
NSTEP = T - 1          # 31 timestep slices used (t=0 gat input, t=1..30 gru inputs)
ROWS = 256             # rows per core
HP = H + 1             # augmented partition count (ones row)
NBLK = N // 128        # 8 j-blocks

_REPO = "/opt/trn_rl_repo"

_cache = {}


def _leaky(v):
    return np.where(v >= 0, v, SLOPE * v)


def _build_module():
    if "nc" in _cache:
        return _cache["nc"]
    if _REPO not in sys.path:
        sys.path.insert(0, _REPO)
    import concourse.bacc as bacc
    import concourse.tile as tile
    from concourse import mybir

    f32 = mybir.dt.float32
    AF = mybir.ActivationFunctionType

    nc = bacc.Bacc("TRN2", target_bir_lowering=False, debug=False, num_devices=NCORES)

    d_xht = nc.dram_tensor("xht", [HP, NSTEP, ROWS], f32, kind="ExternalInput")
    d_gin = nc.dram_tensor("gin", [HP, HP], f32, kind="ExternalInput")
    d_wsel = nc.dram_tensor("wsel", [HP, 2], f32, kind="ExternalInput")
    d_dsel = nc.dram_tensor("dsel", [HP, 2], f32, kind="ExternalInput")
    d_wih = nc.dram_tensor("wih", [HP, 3 * H], f32, kind="ExternalInput")
    d_whh = nc.dram_tensor("whh", [HP, 3 * H], f32, kind="ExternalInput")
    d_wih0 = nc.dram_tensor("wih0", [HP, 3 * H], f32, kind="ExternalInput")
    d_bhhn = nc.dram_tensor("bhhn", [H, 1], f32, kind="ExternalInput")
    d_ident = nc.dram_tensor("ident", [128, 128], f32, kind="ExternalInput")
    d_ones1 = nc.dram_tensor("ones1", [1, 128], f32, kind="ExternalInput")
    d_hout = nc.dram_tensor("hout", [H, ROWS], f32, kind="ExternalOutput")

    TSLICE = HP * ROWS            # fp32 elements of the T-layout payload
    NSLICE = 128 * 130            # fp32 elements of the norm-layout payload
    BLOB = TSLICE + NSLICE
    GROUPS = [[0, 1, 2, 3], [4, 5, 6, 7]]

    with tile.TileContext(nc) as tc:
        with tc.tile_pool(name="persist", bufs=1) as pst, \
             tc.tile_pool(name="masterp", bufs=3) as masterp, \
             tc.tile_pool(name="hstate", bufs=2) as hstate, \
             tc.tile_pool(name="fulls", bufs=2) as fulls, \
             tc.tile_pool(name="ebuf", bufs=2) as ebuf, \
             tc.tile_pool(name="work", bufs=2) as work, \
             tc.tile_pool(name="pays", bufs=2) as pays, \
             tc.tile_pool(name="drams", bufs=2, space="DRAM") as drams, \
             tc.tile_pool(name="psbig", bufs=1, space="PSUM") as psbig, \
             tc.tile_pool(name="psmisc", bufs=4, space="PSUM") as psmisc:

            # ---- load constants / inputs into SBUF ----
            xht = pst.tile([HP, NSTEP, ROWS], f32, tag="xht")
            nc.sync.dma_start(xht[:], d_xht[:])
            gin = pst.tile([HP, HP], f32, tag="gin")
            nc.sync.dma_start(gin[:], d_gin[:])
            wsel = pst.tile([HP, 2], f32, tag="wsel")
            nc.sync.dma_start(wsel[:], d_wsel[:])
            dsel = pst.tile([HP, 2], f32, tag="dsel")
            nc.sync.dma_start(dsel[:], d_dsel[:])
            wih = pst.tile([HP, 3 * H], f32, tag="wih")
            nc.sync.dma_start(wih[:], d_wih[:])
            whh = pst.tile([HP, 3 * H], f32, tag="whh")
            nc.sync.dma_start(whh[:], d_whh[:])
            wih0 = pst.tile([HP, 3 * H], f32, tag="wih0")
            nc.sync.dma_start(wih0[:], d_wih0[:])
            bhhn = pst.tile([H, 1], f32, tag="bhhn")
            nc.sync.dma_start(bhhn[:], d_bhhn[:])
            ident = pst.tile([128, 128], f32, tag="ident")
            nc.sync.dma_start(ident[:], d_ident[:])
            ones1 = pst.tile([1, 128], f32, tag="ones1")
            nc.sync.dma_start(ones1[:], d_ones1[:])

            srow = pst.tile([2, N], f32, tag="srow")    # p0 = s_src, p1 = ones
            drow = pst.tile([2, ROWS], f32, tag="drow")  # p0 = ones, p1 = s_dst

            def gin_step(hsrc):
                """h1T = (h @ g_in_w + b) transposed; ones row via aug col."""
                gp = psmisc.tile([HP, ROWS], f32, tag="ps")
                nc.tensor.matmul(gp[:], gin[:], hsrc, start=True, stop=True)
                m = masterp.tile([HP, ROWS], f32, tag="master")
                nc.vector.tensor_copy(m[:], gp[:])
                return m

            def att_layer(master):
                # --- produce payloads (T-layout direct; norm-layout via PE transpose)
                normsl = pays.tile([128, 130], f32, tag="normsl")
                for half in range(2):
                    pt = psmisc.tile([128, HP], f32, tag="ps")
                    nc.tensor.transpose(
                        pt[:], master[:, 128 * half:128 * (half + 1)],
                        ident[0:HP, 0:HP])
                    nc.vector.tensor_copy(
                        normsl[:, HP * half:HP * (half + 1)], pt[:])

                snd = drams.tile([BLOB], f32, tag="snd")
                nc.sync.dma_start(
                    snd[0:TSLICE].rearrange("(p f) -> p f", p=HP), master[:])
                nc.sync.dma_start(
                    snd[TSLICE:BLOB].rearrange("(p f) -> p f", p=128), normsl[:])
                rcv = drams.tile([4, BLOB], f32, tag="rcv")
                nc.gpsimd.collective_compute(
                    "AllGather", mybir.AluOpType.bypass,
                    ins=[snd[:].opt()], outs=[rcv[:].opt()],
                    replica_groups=GROUPS)

                # --- unpack gathered full state
                h1T = fulls.tile([HP, N], f32, tag="h1T")
                h1n = fulls.tile([128, NBLK, HP], f32, tag="h1n")
                for r in range(4):
                    nc.sync.dma_start(
                        h1T[:, ROWS * r:ROWS * (r + 1)],
                        rcv[r, 0:TSLICE].rearrange("(p f) -> p f", p=HP))
                    nc.sync.dma_start(
                        h1n[:, 2 * r:2 * r + 2, :],
                        rcv[r, TSLICE:BLOB].rearrange(
                            "(p t f) -> p t f", p=128, t=2))

                # --- s_src (full row) and s_dst (own row) + ones rows
                psA0 = psmisc.tile([2, 512], f32, tag="ps")
                psA1 = psmisc.tile([2, 512], f32, tag="ps")
                nc.tensor.matmul(psA0[:], wsel[:], h1T[:, 0:512],
                                 start=True, stop=True)
                nc.tensor.matmul(psA1[:], wsel[:], h1T[:, 512:1024],
                                 start=True, stop=True)
                nc.scalar.copy(srow[:, 0:512], psA0[:])
                nc.scalar.copy(srow[:, 512:1024], psA1[:])
                psB = psmisc.tile([2, ROWS], f32, tag="ps")
                nc.tensor.matmul(psB[:], dsel[:], master[:],
                                 start=True, stop=True)
                nc.vector.tensor_copy(drow[:], psB[:])

                # --- score tiles s[j,i] for all 8 j-blocks into one 4-bank psum
                sps = psbig.tile([128, NBLK, ROWS], f32, tag="sps")
                for k in range(NBLK):
                    nc.tensor.matmul(
                        sps[:, k, :], srow[:, 128 * k:128 * (k + 1)], drow[:],
                        start=True, stop=True)

                # --- e = exp(leaky(s)): two batched ACT passes, no table swap
                lr = ebuf.tile([128, NBLK * ROWS], f32, tag="lr")
                nc.scalar.activation(
                    lr[:].rearrange("p (k f) -> p k f", k=NBLK), sps[:],
                    AF.Prelu, alpha=SLOPE)
                et = ebuf.tile([128, NBLK * ROWS], f32, tag="et")
                nc.scalar.activation(et[:], lr[:], AF.Exp)

                # --- attention contraction + denominator (ones col of h1n)
                attp = psmisc.tile([HP, ROWS], f32, tag="ps")
                for k in range(NBLK):
                    nc.tensor.matmul(
                        attp[:], h1n[:, k, :], et[:, ROWS * k:ROWS * (k + 1)],
                        start=(k == 0), stop=(k == NBLK - 1))

                # --- update: new = att * bcast(1/den) + master
                c1 = work.tile([HP, ROWS], f32, tag="c1")
                nc.scalar.copy(c1[:], attp[:])
                rec = work.tile([1, ROWS], f32, tag="rec")
                nc.vector.reciprocal(rec[:], attp[HP - 1:HP, :])
                Rp = psmisc.tile([HP, ROWS], f32, tag="ps")
                nc.tensor.matmul(Rp[:], ones1[:, 0:HP], rec[:],
                                 start=True, stop=True)
                m1 = work.tile([HP, ROWS], f32, tag="m1")
                nc.vector.tensor_mul(m1[:], c1[:], Rp[:])
                new = masterp.tile([HP, ROWS], f32, tag="master")
                nc.vector.tensor_add(new[0:H, :], m1[0:H, :], master[0:H, :])
                nc.vector.memset(new[H:HP, :], 1.0)
                return new

            def sigmoid(dst, src_ps):
                """dst = 1/(1+exp(-src)); keeps ACT on the exp table set."""
                e = work.tile([H, ROWS], f32, tag="sige")
                nc.scalar.activation(e[:], src_ps, AF.Exp, scale=-1.0)
                t = work.tile([H, ROWS], f32, tag="sigt")
                nc.vector.tensor_scalar_add(t[:], e[:], 1.0)
                nc.vector.reciprocal(dst, t[:])

            def gru(t_idx, out_m, h_old, first):
                """torch GRUCell; inp = xht[t] (or out_m when first), hidden = out_m."""
                WI = wih0 if first else wih
                gi_src = out_m[:] if first else xht[:, t_idx, :]

                rp = psmisc.tile([H, ROWS], f32, tag="ps")
                nc.tensor.matmul(rp[:], WI[:, 0:H], gi_src,
                                 start=True, stop=first)
                if not first:
                    nc.tensor.matmul(rp[:], whh[:, 0:H], out_m[:],
                                     start=False, stop=True)
                r = work.tile([H, ROWS], f32, tag="r")
                sigmoid(r[:], rp[:])

                zp = psmisc.tile([H, ROWS], f32, tag="ps")
                nc.tensor.matmul(zp[:], WI[:, H:2 * H], gi_src,
                                 start=True, stop=first)
                if not first:
                    nc.tensor.matmul(zp[:], whh[:, H:2 * H], out_m[:],
                                     start=False, stop=True)
                z = work.tile([H, ROWS], f32, tag="z")
                sigmoid(z[:], zp[:])

                nip = psmisc.tile([H, ROWS], f32, tag="ps")
                nc.tensor.matmul(nip[:], WI[:, 2 * H:3 * H], gi_src,
                                 start=True, stop=True)
                t1 = work.tile([H, ROWS], f32, tag="t1")
                if first:
                    nc.vector.tensor_scalar_mul(t1[:], r[:], bhhn[:])
                else:
                    hnp = psmisc.tile([H, ROWS], f32, tag="ps")
                    nc.tensor.matmul(hnp[:], whh[:, 2 * H:3 * H], out_m[:],
                                     start=True, stop=True)
                    nc.vector.tensor_mul(t1[:], r[:], hnp[:])
                t2 = work.tile([H, ROWS], f32, tag="t2")
                nc.vector.tensor_add(t2[:], t1[:], nip[:])
                n = work.tile([H, ROWS], f32, tag="n")
                nc.scalar.activation(n[:], t2[:], AF.Tanh)

                hnew = hstate.tile([HP, ROWS], f32, tag="h")
                t3 = work.tile([H, ROWS], f32, tag="t3")
                if first:
                    # h' = (1-z) n
                    nc.vector.tensor_mul(t3[:], z[:], n[:])
                    nc.vector.tensor_sub(hnew[0:H, :], n[:], t3[:])
                else:
                    # h' = n + z (h - n)
                    nc.vector.tensor_sub(t3[:], h_old[0:H, :], n[:])
                    t4 = work.tile([H, ROWS], f32, tag="t4")
                    nc.vector.tensor_mul(t4[:], z[:], t3[:])
                    nc.vector.tensor_add(hnew[0:H, :], n[:], t4[:])
                nc.vector.memset(hnew[H:HP, :], 1.0)
                return hnew

            # ---- the sequential chain ----
            nrun = int(os.environ.get("GAT_STEPS", str(NSTEP)))
            dbg = os.environ.get("GAT_DEBUG", "")
            m = gin_step(xht[:, 0, :])
            if dbg == "gin":
                nc.sync.dma_start(d_hout[:], m[0:H, :])
            elif dbg in ("att1", "att2"):
                m = att_layer(m)
                if dbg == "att2":
                    m = att_layer(m)
                nc.sync.dma_start(d_hout[:], m[0:H, :])
            else:
                m = att_layer(m)
                m = att_layer(m)
                h = gru(0, m, None, first=True)
                for step in range(1, nrun):
                    m = gin_step(h[:])
                    m = att_layer(m)
                    m = att_layer(m)
                    h = gru(step, m, h, first=False)
                nc.sync.dma_start(d_hout[:], h[0:H, :])

    nc.finalize()
    _cache["nc"] = nc
    return nc


def kernel(x, fc_in_w, fc_in_b, g_in_w, g_in_b, g_tr_w, g_tr_b, g_a,
           w_ih, w_hh, b_ih, b_hh, ffn_w, ffn_b, ffn_ow, ffn_ob):
    x = np.asarray(x, np.float32)
    fc_in_w = np.asarray(fc_in_w, np.float32); fc_in_b = np.asarray(fc_in_b, np.float32)
    g_in_w = np.asarray(g_in_w, np.float32); g_in_b = np.asarray(g_in_b, np.float32)
    g_tr_w = np.asarray(g_tr_w, np.float32); g_tr_b = np.asarray(g_tr_b, np.float32)
    g_a = np.asarray(g_a, np.float32)
    w_ih = np.asarray(w_ih, np.float32); w_hh = np.asarray(w_hh, np.float32)
    b_ih = np.asarray(b_ih, np.float32); b_hh = np.asarray(b_hh, np.float32)
    ffn_w = np.asarray(ffn_w, np.float32); ffn_b = np.asarray(ffn_b, np.float32)
    ffn_ow = np.asarray(ffn_ow, np.float32); ffn_ob = np.asarray(ffn_ob, np.float32)

    if _REPO not in sys.path:
        sys.path.insert(0, _REPO)
    from concourse import bass_utils

    # ---- host prep ----
    xp = np.transpose(x, (0, 2, 1, 3))                # [B, N, T, F]
    xh = xp[:, :, 0:NSTEP, :] @ fc_in_w + fc_in_b      # [B, N, 31, H]

    a_src, a_dst = g_a[:H, 0], g_a[H:, 0]
    w_src_aug = np.concatenate([g_tr_w @ a_src, [g_tr_b @ a_src]]).astype(np.float32)
    w_dst_aug = np.concatenate([g_tr_w @ a_dst, [g_tr_b @ a_dst]]).astype(np.float32)
    e64 = np.zeros(HP, np.float32); e64[H] = 1.0
    wsel = np.stack([w_src_aug, e64], axis=1)          # [65, 2]
    dsel = np.stack([e64, w_dst_aug], axis=1)          # [65, 2]

    gin = np.zeros((HP, HP), np.float32)
    gin[0:H, 0:H] = g_in_w
    gin[H, 0:H] = g_in_b
    gin[H, H] = 1.0

    wih = np.concatenate([w_ih.T, b_ih[None, :]], axis=0).astype(np.float32)   # [65,192]
    whh = np.concatenate([w_hh.T, b_hh[None, :]], axis=0).astype(np.float32)
    wih0 = np.concatenate([w_ih.T, (b_ih + b_hh)[None, :]], axis=0).astype(np.float32)
    bhhn = b_hh[2 * H:3 * H].reshape(H, 1).astype(np.float32)
    ident = np.eye(128, dtype=np.float32)
    ones1 = np.ones((1, 128), np.float32)

    shared = dict(gin=gin, wsel=wsel, dsel=dsel, wih=wih, whh=whh, wih0=wih0,
                  bhhn=bhhn, ident=ident, ones1=ones1)
    in_maps = []
    for c in range(NCORES):
        b, s = c // 4, c % 4
        sl = xh[b, ROWS * s:ROWS * (s + 1), :, :]       # [256, 31, 64]
        xht = np.ascontiguousarray(np.transpose(sl, (2, 1, 0)))  # [64, 31, 256]
        xht_aug = np.concatenate(
            [xht, np.ones((1, NSTEP, ROWS), np.float32)], axis=0)
        in_maps.append({"xht": xht_aug, **shared})

    nc = _build_module()
    res = bass_utils.run_bass_kernel_spmd(
        nc, in_maps, core_ids=list(range(NCORES)))

    # ---- assemble h and run the FFN head on host ----
    hfull = np.zeros((B, N, H), np.float32)
    for c in range(NCORES):
        b, s = c // 4, c % 4
        hfull[b, ROWS * s:ROWS * (s + 1), :] = res.results[c]["hout"].T
    hid = _leaky(hfull @ ffn_w + ffn_b)
    return ((hid @ ffn_ow + ffn_ob)[..., 0]).astype(np.float32)
